# revision 3
# baseline (speedup 1.0000x reference)
"""DeepSeek-style GQA attention block (B=2, S=2048, H=1536, 12 q-heads /
2 kv-heads, d=128) sharded over 8 TRN2 NeuronCores.

Sharding: core = (batch b, kv-group hh, query-half th).
  - tensor parallel over the 2 kv groups (6 q-heads + 1 kv head each)
  - data parallel over batch (2)
  - query-token parallel (2 halves of 1024)
Each core computes its 6 heads' attention for its 1024 query tokens against
the full 2048-token K/V of its kv head, then a partial O-projection; the two
kv-group partials per (b, th) are summed on the host.

All matmuls run in bf16 with fp32 PSUM accumulation. Softmax runs without
max-subtraction (scores are O(1) here), with the 1/sqrt(d) scale and the
additive attention-mask bias fused into the ACT exp instruction.

Layout trick: scores are computed TRANSPOSED (scores^T[Sk, Sq] = K Q^T) so
that the attention probabilities come out with Sk on partitions, which is
exactly the layout the AV matmul needs as its moving operand — no on-chip
transpose of the [Sk, Sq] probability matrix is ever needed.

The softmax denominator (a partition-dim reduction over all 2048 keys) is
kept OFF the tensor engine: exp chunks are accumulated on the DVE (bf16
adds), the 128-partition reduction+broadcast runs on the otherwise-idle
GpSimd engine (partition_all_reduce), and normalization is a DVE
reciprocal/multiply. This removes 192 rowsum matmuls + 12 broadcast matmuls
per core versus the naive scheme.

The O-projection is split into three per-head-group passes (h0-2, h3-4, h5)
accumulated through an f32 SBUF partial. Pass work is enqueued as deferred
units and pumped one unit per chunk-iteration inside the later heads'
attention loops: the tensor engine stays continuously busy (TRN2's PE
p-state throttle only reaches 2.4 GHz after 3us of uninterrupted work), and
only head 5's single-matmul pass remains after the last exp.
"""

import numpy as np
import ml_dtypes

HIDDEN = 1536
D = 128          # head dim
NH = 6           # q-heads per core
B, S = 2, 2048
SQ = 1024        # query tokens per core
HC = HIDDEN // 128   # 12 hidden chunks
SKC = S // 128       # 16 key chunks
SCALE = float(1.0 / np.sqrt(np.float32(D)))

_NC_CACHE = {}
last_results = None  # BassKernelResults of the most recent run (for test.py)


def _build_nc():
    import concourse.bacc as bacc
    import concourse.mybir as mybir
    import concourse.tile as tile
    import concourse.bass_isa as bass_isa
    from concourse.masks import make_identity

    bf16 = mybir.dt.bfloat16
    f32 = mybir.dt.float32
    Exp = mybir.ActivationFunctionType.Exp
    Copy = mybir.ActivationFunctionType.Copy
    RAdd = bass_isa.ReduceOp.add

    nc = bacc.Bacc("TRN2", target_bir_lowering=False, debug=False, num_devices=8)

    xt = nc.dram_tensor("xt", [HIDDEN, S], bf16, kind="ExternalInput")
    wqt = nc.dram_tensor("wqt", [HIDDEN, NH * D], bf16, kind="ExternalInput")
    wkt = nc.dram_tensor("wkt", [HIDDEN, D], bf16, kind="ExternalInput")
    wvt = nc.dram_tensor("wvt", [HIDDEN, D], bf16, kind="ExternalInput")
    wot = nc.dram_tensor("wot", [NH * D, HIDDEN], bf16, kind="ExternalInput")
    biasd = nc.dram_tensor("biasd", [128, SKC], f32, kind="ExternalInput")
    y = nc.dram_tensor("y", [SQ, HIDDEN], bf16, kind="ExternalOutput")

    NB = HIDDEN // 512   # 3 o-proj column blocks
    NT = SQ // 128       # 8 o-proj token blocks

    with tile.TileContext(nc) as tc:
        with (
            tc.tile_pool(name="const", bufs=1) as constp,
            tc.tile_pool(name="weights", bufs=1) as wp,
            tc.tile_pool(name="persist", bufs=1) as pers,
        ):
            ident = constp.tile([128, 128], bf16)
            make_identity(nc, ident[:])
            bias_sb = constp.tile([128, SKC], f32)

            wk_sb = wp.tile([128, HC, D], bf16)
            wv_sb = wp.tile([128, HC, D], bf16)
            wo_sb = wp.tile([128, NH, HIDDEN], bf16)

            kT_sb = pers.tile([128, S], bf16)         # K^T [d, Sk]
            v_sb = pers.tile([128, SKC, D], bf16)     # V [Sk, d], chunked
            qT_sb = pers.tile([128, NH, SQ], bf16)    # Q^T [d, Sq] per head
            outT_sb = pers.tile([128, NH, SQ], bf16)  # normalized AV out^T
            # o-proj partial sums, one [128, 512] f32 tile per (t, nb)
            part_sb = pers.tile([128, NT * NB, 512], f32)

            # ---------- phase 1-3: projections ----------
            with tc.tile_pool(name="xtp", bufs=1) as xtp:
                xt_sb = xtp.tile([128, HC, S], bf16)
                wq_sb = xtp.tile([128, HC, NH * D], bf16)
                vT_sb = xtp.tile([128, S], bf16)

                # DMA order is the early-pipeline schedule: the first xt
                # chunk lands before anything else so the first K/V matmul
                # can issue ~9us in; wq chunks interleave into the tail of
                # the xt stream (Q-proj starts after KV-proj, so they can
                # arrive late); wo/bias are needed far later.
                wqt_r = wqt.ap().rearrange("(c p) m -> p c m", p=128)
                nc.sync.dma_start(xt_sb[:, 0, :], xt[0:128, :])
                nc.sync.dma_start(wk_sb[:], wkt.ap().rearrange("(c p) m -> p c m", p=128))
                nc.sync.dma_start(wv_sb[:], wvt.ap().rearrange("(c p) m -> p c m", p=128))
                wq_queue = list(range(HC))
                for c in range(1, HC):
                    nc.sync.dma_start(xt_sb[:, c, :], xt[128 * c : 128 * (c + 1), :])
                    if c >= 5:
                        cq = wq_queue.pop(0)
                        nc.sync.dma_start(wq_sb[:, cq, :], wqt_r[:, cq, :])
                for cq in wq_queue:
                    nc.sync.dma_start(wq_sb[:, cq, :], wqt_r[:, cq, :])
                nc.sync.dma_start(bias_sb[:], biasd.ap())
                nc.sync.dma_start(
                    wo_sb[:], wot.ap().rearrange("(h p) n -> p h n", p=128)
                )

                # K^T and V^T = W X^T accumulate chunk-major so every xt
                # chunk arrival feeds 8 matmuls immediately (all 8 PSUM
                # banks; the pool closes before the transpose/Q pool opens).
                with tc.tile_pool(name="kv_ps", bufs=1, space="PSUM") as kvps:
                    kps = kvps.tile([128, 4, 512], f32, tag="kps")
                    vps = kvps.tile([128, 4, 512], f32, tag="vps")
                    for c in range(HC):
                        for w_sb, ps in ((wk_sb, kps), (wv_sb, vps)):
                            for sb in range(S // 512):
                                nc.tensor.matmul(
                                    ps[:, sb, :],
                                    w_sb[:, c, :],
                                    xt_sb[:, c, 512 * sb : 512 * (sb + 1)],
                                    start=(c == 0),
                                    stop=(c == HC - 1),
                                )
                    # v casts first: the V transposes depend on them
                    for ps, dst in ((vps, vT_sb), (kps, kT_sb)):
                        for sb in range(S // 512):
                            nc.vector.tensor_copy(
                                dst[:, 512 * sb : 512 * (sb + 1)], ps[:, sb, :]
                            )

                with tc.tile_pool(name="proj_ps", bufs=2, space="PSUM") as pps:
                    def emit_q(h):
                        ps = pps.tile([128, SQ], f32, tag="projq")
                        for sqh in range(2):
                            for c in range(HC):
                                nc.tensor.matmul(
                                    ps[:, 512 * sqh : 512 * (sqh + 1)],
                                    wq_sb[:, c, D * h : D * (h + 1)],
                                    xt_sb[:, c, 512 * sqh : 512 * (sqh + 1)],
                                    start=(c == 0),
                                    stop=(c == HC - 1),
                                )
                        nc.vector.tensor_copy(qT_sb[:, h, :], ps[:])

                    # Q head 0 first: it has no dependency on the PSUM->SBUF
                    # casts above, so the PE rolls straight from the KV
                    # matmuls into it while the DVE drains the casts; the V
                    # transposes (which need the vT casts) come after.
                    emit_q(0)
                    for c in range(SKC):
                        pt = pps.tile([128, 128], bf16, tag="vtr")
                        nc.tensor.transpose(
                            pt[:], vT_sb[:, 128 * c : 128 * (c + 1)], ident[:]
                        )
                        nc.vector.tensor_copy(v_sb[:, c, :], pt[:])
                    for h in range(1, NH):
                        emit_q(h)

            # ---------- phase 4: attention + pipelined o-projection ----------
            with (
                tc.tile_pool(name="sc_ps", bufs=2, space="PSUM") as scp,
                tc.tile_pool(name="av_ps", bufs=1, space="PSUM") as avp,
                tc.tile_pool(name="y_ps", bufs=2, space="PSUM") as yp,
                tc.tile_pool(name="esb", bufs=6) as ep,
                tc.tile_pool(name="eaccp", bufs=2) as eaccp,
                tc.tile_pool(name="rsp", bufs=2) as rsp,
                tc.tile_pool(name="avsb", bufs=2) as avsbp,
                tc.tile_pool(name="y_sb", bufs=3) as ysb,
            ):
                # deferred o-proj pass units, pumped into the chunk loops
                unit_q = []

                def pump(n):
                    for _ in range(min(n, len(unit_q))):
                        unit_q.pop(0)()

                def opass_unit(u, heads, first, last):
                    t, nb = divmod(u, NB)

                    def run():
                        ps = yp.tile([128, 512], f32, tag="y")
                        for i, h in enumerate(heads):
                            nc.tensor.matmul(
                                ps[:],
                                outT_sb[:, h, 128 * t : 128 * (t + 1)],
                                wo_sb[:, h, 512 * nb : 512 * (nb + 1)],
                                start=(i == 0),
                                stop=(i == len(heads) - 1),
                            )
                        if first:
                            nc.vector.tensor_copy(part_sb[:, u, :], ps[:])
                        elif not last:
                            nc.vector.tensor_add(
                                part_sb[:, u, :], part_sb[:, u, :], ps[:]
                            )
                        else:
                            ysb_t = ysb.tile([128, 512], bf16, tag="ysb")
                            nc.vector.tensor_add(ysb_t[:], part_sb[:, u, :], ps[:])
                            nc.sync.dma_start(
                                y[128 * t : 128 * (t + 1), 512 * nb : 512 * (nb + 1)],
                                ysb_t[:],
                            )

                    return run

                for h in range(NH):
                    av = avp.tile([128, SQ], f32, tag="av")
                    eacc = eaccp.tile([128, SQ], bf16, tag="eacc")
                    e_tiles = {}

                    def emit_scores(c):
                        # scores^T chunk [Sk 128, Sq 1024] = (K^T slice)^T Q^T
                        sc = scp.tile([128, SQ], f32, tag="sc")
                        for sqh in range(2):
                            nc.tensor.matmul(
                                sc[:, 512 * sqh : 512 * (sqh + 1)],
                                kT_sb[:, 128 * c : 128 * (c + 1)],
                                qT_sb[:, h, 512 * sqh : 512 * (sqh + 1)],
                                start=True,
                                stop=True,
                            )
                        # e = exp(scale * scores + mask_bias)  (bias is per-Sk
                        # = per-partition, exactly what ACT bias supports)
                        et = ep.tile([128, SQ], bf16, tag="e")
                        nc.scalar.activation(
                            et[:], sc[:], Exp,
                            bias=bias_sb[:, c : c + 1], scale=SCALE,
                        )
                        e_tiles[c] = et

                    def emit_av_acc(c):
                        et = e_tiles.pop(c)
                        for sqh in range(2):
                            nc.tensor.matmul(
                                av[:, 512 * sqh : 512 * (sqh + 1)],
                                v_sb[:, c, :],
                                et[:, 512 * sqh : 512 * (sqh + 1)],
                                start=(c == 0),
                                stop=(c == SKC - 1),
                            )
                        # denominator accumulation stays off the PE
                        if c == 0:
                            nc.vector.tensor_copy(eacc[:], et[:])
                        else:
                            nc.vector.tensor_add(eacc[:], eacc[:], et[:])

                    emit_scores(0)
                    for c in range(SKC):
                        if c + 1 < SKC:
                            emit_scores(c + 1)
                        emit_av_acc(c)
                        pump(2 if h == NH - 1 else 1)

                    # head tail: 128-way key reduction on GpSimd (broadcast
                    # all-reduce), reciprocal + normalize on DVE. The ACT
                    # copy moves av out of PSUM so the next head's AV
                    # accumulation can start during this chain.
                    rs = rsp.tile([128, SQ], f32, tag="rs")
                    nc.gpsimd.partition_all_reduce(rs[:], eacc[:], 128, RAdd)
                    brec = rsp.tile([128, SQ], f32, tag="brec")
                    nc.vector.reciprocal_approx_fast(brec[:], rs[:])
                    av_f = avsbp.tile([128, SQ], f32, tag="avf")
                    nc.scalar.activation(av_f[:], av[:], Copy)
                    nc.vector.tensor_mul(outT_sb[:, h, :], av_f[:], brec[:])

                    if h == 2:
                        for u in range(NT * NB):
                            unit_q.append(opass_unit(u, (0, 1, 2), True, False))
                    elif h == 4:
                        for u in range(NT * NB):
                            unit_q.append(opass_unit(u, (3, 4), False, False))

                # leftovers, then the final single-matmul pass for head 5
                pump(len(unit_q))
                for u in range(NT * NB):
                    opass_unit(u, (5,), False, True)()

    nc.compile()
    return nc


def _get_nc():
    if "nc" not in _NC_CACHE:
        _NC_CACHE["nc"] = _build_nc()
    return _NC_CACHE["nc"]


def kernel(hidden_states, attention_mask, Wq, Wk, Wv, Wo):
    global last_results
    from concourse.bass_utils import run_bass_kernel_spmd

    bf = ml_dtypes.bfloat16
    hidden_states = np.asarray(hidden_states, dtype=np.float32)
    attention_mask = np.asarray(attention_mask, dtype=np.float32)
    Wq = np.asarray(Wq, dtype=np.float32)
    Wk = np.asarray(Wk, dtype=np.float32)
    Wv = np.asarray(Wv, dtype=np.float32)
    Wo = np.asarray(Wo, dtype=np.float32)

    nc = _get_nc()

    in_maps = []
    cores = []
    for b in range(2):
        xt_full = np.ascontiguousarray(hidden_states[b].T).astype(bf)  # [H, S]
        bias_full = ((1.0 - attention_mask[b]) * -10000.0).astype(np.float32)
        for hh in range(2):
            wqt = np.ascontiguousarray(
                Wq[NH * D * hh : NH * D * (hh + 1), :].T
            ).astype(bf)
            wkt = np.ascontiguousarray(Wk[D * hh : D * (hh + 1), :].T).astype(bf)
            wvt = np.ascontiguousarray(Wv[D * hh : D * (hh + 1), :].T).astype(bf)
            wot = np.ascontiguousarray(
                Wo[:, NH * D * hh : NH * D * (hh + 1)].T
            ).astype(bf)
            for th in range(2):
                # roll tokens so this core's queries are columns 0..SQ-1
                r = th * SQ
                xt_r = np.ascontiguousarray(
                    np.concatenate([xt_full[:, r:], xt_full[:, :r]], axis=1)
                )
                bias_r = np.concatenate([bias_full[r:], bias_full[:r]])
                biasd = np.ascontiguousarray(
                    bias_r.reshape(SKC, 128).T
                ).astype(np.float32)
                in_maps.append(
                    {
                        "xt": xt_r,
                        "wqt": wqt,
                        "wkt": wkt,
                        "wvt": wvt,
                        "wot": wot,
                        "biasd": biasd,
                    }
                )
                cores.append((b, hh, th))

    res = run_bass_kernel_spmd(nc, in_maps, core_ids=list(range(8)))
    last_results = res

    out = np.zeros((B, S, HIDDEN), dtype=np.float32)
    for (b, hh, th), r in zip(cores, res.results):
        out[b, th * SQ : (th + 1) * SQ, :] += np.asarray(r["y"], dtype=np.float32)
    return out


# revision 4
# speedup vs baseline: 1.2143x; 1.2143x over previous
"""DeepSeek-style GQA attention block (B=2, S=2048, H=1536, 12 q-heads /
2 kv-heads, d=128) sharded over 8 TRN2 NeuronCores.

Sharding: core = (batch b, kv-group hh, query-half th).
  - tensor parallel over the 2 kv groups (6 q-heads + 1 kv head each)
  - data parallel over batch (2)
  - query-token parallel (2 halves of 1024)
Each core computes its 6 heads' attention for its 1024 query tokens against
the full 2048-token K/V of its kv head, then a partial O-projection; the two
kv-group partials per (b, th) are summed on the host.

All matmuls run in bf16 with fp32 PSUM accumulation. Softmax runs without
max-subtraction (scores are O(1) here), with the 1/sqrt(d) scale and the
additive attention-mask bias fused into the ACT exp instruction.

Layout trick: scores are computed TRANSPOSED (scores^T[Sk, Sq] = K Q^T) so
the attention probabilities come out with Sk on partitions, which is exactly
the layout the AV matmul needs as its moving operand.

Softmax denominator: exp chunks are accumulated on the DVE (bf16 adds, with
the last chunk folded directly into the reduction matmul), then ONE ones-
vector matmul per head-half does the 128-partition reduction, a rank-1
matmul broadcasts it back over the d partitions, and DVE reciprocal/multiply
normalizes the AV output. This costs the PE ~1.3us/head instead of the
~8us/head of a per-chunk rowsum scheme, and keeps the slow GpSimd engine
(partition_all_reduce measured 6.7us!) off the critical path entirely.

Scheduling: TRN2's PE p-state throttle only reaches 2.4 GHz after 3us of
uninterrupted work, so the kernel keeps the PE queue dense. All deferrable
matmul work — Q-projection for heads 1-5 and the first two O-projection
passes (h0-2, h3-4, merged through a bf16 SBUF partial) — is enqueued as
closures and pumped one unit per chunk-iteration inside the attention
loops, filling what would otherwise be exp-wait bubbles. After head 5 only
a single-matmul O-pass remains: partial + h5 contribution accumulate in
PSUM via an identity matmul, with writebacks alternating ACT/DVE.
"""

import numpy as np
import ml_dtypes

HIDDEN = 1536
D = 128          # head dim
NH = 6           # q-heads per core
B, S = 2, 2048
SQ = 1024        # query tokens per core
HC = HIDDEN // 128   # 12 hidden chunks
SKC = S // 128       # 16 key chunks
SCALE = float(1.0 / np.sqrt(np.float32(D)))

_NC_CACHE = {}
last_results = None  # BassKernelResults of the most recent run (for test.py)


def _build_nc():
    import concourse.bacc as bacc
    import concourse.mybir as mybir
    import concourse.tile as tile
    from concourse.masks import make_identity

    bf16 = mybir.dt.bfloat16
    f32 = mybir.dt.float32
    Exp = mybir.ActivationFunctionType.Exp
    Copy = mybir.ActivationFunctionType.Copy

    nc = bacc.Bacc("TRN2", target_bir_lowering=False, debug=False, num_devices=8)

    xt = nc.dram_tensor("xt", [HIDDEN, S], bf16, kind="ExternalInput")
    wqt = nc.dram_tensor("wqt", [HIDDEN, NH * D], bf16, kind="ExternalInput")
    wkt = nc.dram_tensor("wkt", [HIDDEN, D], bf16, kind="ExternalInput")
    wvt = nc.dram_tensor("wvt", [HIDDEN, D], bf16, kind="ExternalInput")
    wot = nc.dram_tensor("wot", [NH * D, HIDDEN], bf16, kind="ExternalInput")
    biasd = nc.dram_tensor("biasd", [128, SKC], f32, kind="ExternalInput")
    y = nc.dram_tensor("y", [SQ, HIDDEN], bf16, kind="ExternalOutput")

    NB = HIDDEN // 512   # 3 o-proj column blocks
    NT = SQ // 128       # 8 o-proj token blocks
    NU = NT * NB         # 24 o-proj tiles

    with tile.TileContext(nc) as tc:
        with (
            tc.tile_pool(name="const", bufs=1) as constp,
            tc.tile_pool(name="weights", bufs=1) as wp,
            tc.tile_pool(name="persist", bufs=1) as pers,
        ):
            ident = constp.tile([128, 128], bf16)
            make_identity(nc, ident[:])
            ones_col = constp.tile([128, 1], bf16)
            nc.vector.memset(ones_col[:], 1.0)
            ones_row = constp.tile([1, 128], bf16)
            nc.vector.memset(ones_row[:], 1.0)
            bias_sb = constp.tile([128, SKC], f32)

            wk_sb = wp.tile([128, HC, D], bf16)
            wv_sb = wp.tile([128, HC, D], bf16)
            wo_sb = wp.tile([128, NH, HIDDEN], bf16)

            kT_sb = pers.tile([128, S], bf16)         # K^T [d, Sk]
            v_sb = pers.tile([128, SKC, D], bf16)     # V [Sk, d], chunked
            qT_sb = pers.tile([128, NH, SQ], bf16)    # Q^T [d, Sq] per head
            outT_sb = pers.tile([128, NH, SQ], bf16)  # normalized AV out^T
            part_sb = pers.tile([128, NU, 512], bf16) # o-proj partials
            xt_sb = pers.tile([128, HC, S], bf16)
            wq_sb = pers.tile([128, HC, NH * D], bf16)
            vT_sb = pers.tile([128, S], bf16)

            # DMA order is the early-pipeline schedule: first xt chunk, then
            # the small K/V weights (first matmul needs them), mask bias,
            # then the xt stream with wq interleaved into its tail, wo last.
            wqt_r = wqt.ap().rearrange("(c p) m -> p c m", p=128)
            nc.sync.dma_start(xt_sb[:, 0, :], xt[0:128, :])
            nc.sync.dma_start(wk_sb[:], wkt.ap().rearrange("(c p) m -> p c m", p=128))
            nc.sync.dma_start(wv_sb[:], wvt.ap().rearrange("(c p) m -> p c m", p=128))
            nc.sync.dma_start(bias_sb[:], biasd.ap())
            wq_queue = list(range(HC))
            for c in range(1, HC):
                nc.sync.dma_start(xt_sb[:, c, :], xt[128 * c : 128 * (c + 1), :])
                if c >= 5:
                    cq = wq_queue.pop(0)
                    nc.sync.dma_start(wq_sb[:, cq, :], wqt_r[:, cq, :])
            for cq in wq_queue:
                nc.sync.dma_start(wq_sb[:, cq, :], wqt_r[:, cq, :])
            nc.sync.dma_start(
                wo_sb[:], wot.ap().rearrange("(h p) n -> p h n", p=128)
            )

            # ---------- K/V projection ----------
            with tc.tile_pool(name="kv_ps", bufs=1, space="PSUM") as kvps:
                kps = kvps.tile([128, 4, 512], f32, tag="kps")
                vps = kvps.tile([128, 4, 512], f32, tag="vps")
                for c in range(HC):
                    for w_sb, ps in ((wk_sb, kps), (wv_sb, vps)):
                        for sb in range(S // 512):
                            nc.tensor.matmul(
                                ps[:, sb, :],
                                w_sb[:, c, :],
                                xt_sb[:, c, 512 * sb : 512 * (sb + 1)],
                                start=(c == 0),
                                stop=(c == HC - 1),
                            )
                # v casts first: the V transposes depend on them
                for ps, dst in ((vps, vT_sb), (kps, kT_sb)):
                    for sb in range(S // 512):
                        nc.vector.tensor_copy(
                            dst[:, 512 * sb : 512 * (sb + 1)], ps[:, sb, :]
                        )

            # ---------- Q head 0 + V transpose (PE rolls on, casts on DVE) --
            with tc.tile_pool(name="proj_ps", bufs=2, space="PSUM") as pps:
                ps = pps.tile([128, SQ], f32, tag="projq")
                for sqh in range(2):
                    for c in range(HC):
                        nc.tensor.matmul(
                            ps[:, 512 * sqh : 512 * (sqh + 1)],
                            wq_sb[:, c, 0:D],
                            xt_sb[:, c, 512 * sqh : 512 * (sqh + 1)],
                            start=(c == 0),
                            stop=(c == HC - 1),
                        )
                nc.vector.tensor_copy(qT_sb[:, 0, :], ps[:])
                for c in range(SKC):
                    pt = pps.tile([128, 128], bf16, tag="vtr")
                    nc.tensor.transpose(
                        pt[:], vT_sb[:, 128 * c : 128 * (c + 1)], ident[:]
                    )
                    nc.vector.tensor_copy(v_sb[:, c, :], pt[:])

            # ---------- attention + pipelined projections ----------
            with (
                tc.tile_pool(name="sc_ps", bufs=2, space="PSUM") as scp,
                tc.tile_pool(name="av_ps", bufs=1, space="PSUM") as avp,
                tc.tile_pool(name="y_ps", bufs=2, space="PSUM") as yp,
                tc.tile_pool(name="esb", bufs=8) as ep,
                tc.tile_pool(name="eaccp", bufs=2) as eaccp,
                tc.tile_pool(name="rowp", bufs=4) as rowp,
                tc.tile_pool(name="brecp", bufs=2) as brecp,
                tc.tile_pool(name="avsb", bufs=2) as avsbp,
                tc.tile_pool(name="y_sb", bufs=3) as ysb,
            ):
                # deferred PE work: ('q', head, fn) or ('o', fn)
                unit_q = []

                def pump(n):
                    for _ in range(min(n, len(unit_q))):
                        unit_q.pop(0)[-1]()

                def drain_q(h):
                    while unit_q and unit_q[0][0] == "q" and unit_q[0][1] <= h:
                        unit_q.pop(0)[-1]()

                def q_stream(qh, sqh):
                    def run():
                        ps = yp.tile([128, 512], f32, tag="y")
                        for c in range(HC):
                            nc.tensor.matmul(
                                ps[:],
                                wq_sb[:, c, D * qh : D * (qh + 1)],
                                xt_sb[:, c, 512 * sqh : 512 * (sqh + 1)],
                                start=(c == 0),
                                stop=(c == HC - 1),
                            )
                        nc.vector.tensor_copy(
                            qT_sb[:, qh, 512 * sqh : 512 * (sqh + 1)], ps[:]
                        )

                    return run

                def opass_unit(u, heads, first):
                    t, nb = divmod(u, NB)

                    def run():
                        ps = yp.tile([128, 512], f32, tag="y")
                        for i, h in enumerate(heads):
                            nc.tensor.matmul(
                                ps[:],
                                outT_sb[:, h, 128 * t : 128 * (t + 1)],
                                wo_sb[:, h, 512 * nb : 512 * (nb + 1)],
                                start=(i == 0),
                                stop=(i == len(heads) - 1),
                            )
                        if first:
                            nc.vector.tensor_copy(part_sb[:, u, :], ps[:])
                        else:
                            nc.vector.tensor_add(
                                part_sb[:, u, :], part_sb[:, u, :], ps[:]
                            )

                    return run

                for qh in range(1, NH):
                    for sqh in range(2):
                        unit_q.append(("q", qh, q_stream(qh, sqh)))

                for h in range(NH):
                    drain_q(h + 1)
                    av = avp.tile([128, SQ], f32, tag="av")
                    eacc = eaccp.tile([128, SQ], bf16, tag="eacc")
                    e_tiles = {}

                    def emit_scores(c):
                        # scores^T chunk [Sk 128, Sq 1024] = (K^T slice)^T Q^T
                        sc = scp.tile([128, SQ], f32, tag="sc")
                        for sqh in range(2):
                            nc.tensor.matmul(
                                sc[:, 512 * sqh : 512 * (sqh + 1)],
                                kT_sb[:, 128 * c : 128 * (c + 1)],
                                qT_sb[:, h, 512 * sqh : 512 * (sqh + 1)],
                                start=True,
                                stop=True,
                            )
                        et = ep.tile([128, SQ], bf16, tag="e")
                        nc.scalar.activation(
                            et[:], sc[:], Exp,
                            bias=bias_sb[:, c : c + 1], scale=SCALE,
                        )
                        e_tiles[c] = et

                    def emit_av_acc(c):
                        et = e_tiles[c]
                        for sqh in range(2):
                            nc.tensor.matmul(
                                av[:, 512 * sqh : 512 * (sqh + 1)],
                                v_sb[:, c, :],
                                et[:, 512 * sqh : 512 * (sqh + 1)],
                                start=(c == 0),
                                stop=(c == SKC - 1),
                            )
                        # denominator accumulation stays off the PE; the last
                        # chunk is folded into the reduction matmul instead
                        if c == 0:
                            nc.vector.tensor_copy(eacc[:], et[:])
                        elif c < SKC - 1:
                            nc.vector.tensor_add(eacc[:], eacc[:], et[:])

                    emit_scores(0)
                    for c in range(SKC):
                        if c + 1 < SKC:
                            emit_scores(c + 1)
                        emit_av_acc(c)
                        pump(1)

                    # head tail: ones-matmul partition reduction of the
                    # denominator (+ last exp chunk), rank-1 broadcast,
                    # reciprocal + normalize. ACT copies av out of PSUM so
                    # the next head's AV accumulation isn't gated on this.
                    e15 = e_tiles[SKC - 1]
                    av_f = avsbp.tile([128, SQ], f32, tag="avf")
                    nc.scalar.activation(av_f[:], av[:], Copy)
                    for sqh in range(2):
                        sl = slice(512 * sqh, 512 * (sqh + 1))
                        rs = yp.tile([128, 512], f32, tag="y")
                        nc.tensor.matmul(
                            rs[0:1, :], ones_col[:], eacc[:, sl],
                            start=True, stop=False,
                        )
                        nc.tensor.matmul(
                            rs[0:1, :], ones_col[:], e15[:, sl],
                            start=False, stop=True,
                        )
                        row = rowp.tile([1, 512], bf16, tag="row")
                        nc.vector.tensor_copy(row[:], rs[0:1, :])
                        bc = yp.tile([128, 512], f32, tag="y")
                        nc.tensor.matmul(
                            bc[:], ones_row[:], row[:], start=True, stop=True
                        )
                        brec = brecp.tile([128, 512], f32, tag="brec")
                        nc.vector.reciprocal_approx_fast(brec[:], bc[:])
                        nc.vector.tensor_mul(
                            outT_sb[:, h, sl], av_f[:, sl], brec[:]
                        )

                    if h == 2:
                        for u in range(NU):
                            unit_q.append(("o", opass_unit(u, (0, 1, 2), True)))
                    elif h == 4:
                        for u in range(NU):
                            unit_q.append(("o", opass_unit(u, (3, 4), False)))

                # drain leftovers, then the final o-pass: bf16 partial folded
                # back into PSUM via an identity matmul + h5's contribution;
                # writebacks alternate ACT/DVE to halve the drain tail.
                pump(len(unit_q))
                for u in range(NU):
                    t, nb = divmod(u, NB)
                    ps = yp.tile([128, 512], f32, tag="y")
                    nc.tensor.matmul(
                        ps[:], ident[:], part_sb[:, u, :], start=True, stop=False
                    )
                    nc.tensor.matmul(
                        ps[:],
                        outT_sb[:, 5, 128 * t : 128 * (t + 1)],
                        wo_sb[:, 5, 512 * nb : 512 * (nb + 1)],
                        start=False,
                        stop=True,
                    )
                    ysb_t = ysb.tile([128, 512], bf16, tag="ysb")
                    if u % 2 == 0:
                        nc.scalar.activation(ysb_t[:], ps[:], Copy)
                    else:
                        nc.vector.tensor_copy(ysb_t[:], ps[:])
                    nc.sync.dma_start(
                        y[128 * t : 128 * (t + 1), 512 * nb : 512 * (nb + 1)],
                        ysb_t[:],
                    )

    nc.compile()
    return nc


def _get_nc():
    if "nc" not in _NC_CACHE:
        _NC_CACHE["nc"] = _build_nc()
    return _NC_CACHE["nc"]


def kernel(hidden_states, attention_mask, Wq, Wk, Wv, Wo):
    global last_results
    from concourse.bass_utils import run_bass_kernel_spmd

    bf = ml_dtypes.bfloat16
    hidden_states = np.asarray(hidden_states, dtype=np.float32)
    attention_mask = np.asarray(attention_mask, dtype=np.float32)
    Wq = np.asarray(Wq, dtype=np.float32)
    Wk = np.asarray(Wk, dtype=np.float32)
    Wv = np.asarray(Wv, dtype=np.float32)
    Wo = np.asarray(Wo, dtype=np.float32)

    nc = _get_nc()

    in_maps = []
    cores = []
    for b in range(2):
        xt_full = np.ascontiguousarray(hidden_states[b].T).astype(bf)  # [H, S]
        bias_full = ((1.0 - attention_mask[b]) * -10000.0).astype(np.float32)
        for hh in range(2):
            wqt = np.ascontiguousarray(
                Wq[NH * D * hh : NH * D * (hh + 1), :].T
            ).astype(bf)
            wkt = np.ascontiguousarray(Wk[D * hh : D * (hh + 1), :].T).astype(bf)
            wvt = np.ascontiguousarray(Wv[D * hh : D * (hh + 1), :].T).astype(bf)
            wot = np.ascontiguousarray(
                Wo[:, NH * D * hh : NH * D * (hh + 1)].T
            ).astype(bf)
            for th in range(2):
                # roll tokens so this core's queries are columns 0..SQ-1
                r = th * SQ
                xt_r = np.ascontiguousarray(
                    np.concatenate([xt_full[:, r:], xt_full[:, :r]], axis=1)
                )
                bias_r = np.concatenate([bias_full[r:], bias_full[:r]])
                biasd = np.ascontiguousarray(
                    bias_r.reshape(SKC, 128).T
                ).astype(np.float32)
                in_maps.append(
                    {
                        "xt": xt_r,
                        "wqt": wqt,
                        "wkt": wkt,
                        "wvt": wvt,
                        "wot": wot,
                        "biasd": biasd,
                    }
                )
                cores.append((b, hh, th))

    res = run_bass_kernel_spmd(nc, in_maps, core_ids=list(range(8)))
    last_results = res

    out = np.zeros((B, S, HIDDEN), dtype=np.float32)
    for (b, hh, th), r in zip(cores, res.results):
        out[b, th * SQ : (th + 1) * SQ, :] += np.asarray(r["y"], dtype=np.float32)
    return out


# revision 11
# speedup vs baseline: 1.2210x; 1.0056x over previous
"""DeepSeek-style GQA attention block (B=2, S=2048, H=1536, 12 q-heads /
2 kv-heads, d=128) sharded over 8 TRN2 NeuronCores.

Sharding: core = (batch b, kv-group hh, query-half th).
  - tensor parallel over the 2 kv groups (6 q-heads + 1 kv head each)
  - data parallel over batch (2)
  - query-token parallel (2 halves of 1024)
Each core computes its 6 heads' attention for its 1024 query tokens against
the full 2048-token K/V of its kv head, then a partial O-projection; the two
kv-group partials per (b, th) are summed on the host.

All matmuls run in bf16 with fp32 PSUM accumulation. Softmax runs without
max-subtraction (scores are O(1) here), with the 1/sqrt(d) scale and the
additive attention-mask bias fused into the ACT exp instruction.

Layout trick: scores are computed TRANSPOSED (scores^T[Sk, Sq] = K Q^T) so
the attention probabilities come out with Sk on partitions, which is exactly
the layout the AV matmul needs as its moving operand.

Softmax denominator: exp chunks are accumulated on the DVE (bf16 adds, with
the last chunk folded directly into the reduction matmul), then ONE ones-
vector matmul per head-half does the 128-partition reduction, a rank-1
matmul broadcasts it back over the d partitions, and DVE reciprocal/multiply
normalizes the AV output. This costs the PE ~1.3us/head instead of the
~8us/head of a per-chunk rowsum scheme, and keeps the slow GpSimd engine
(partition_all_reduce measured 6.7us!) off the critical path entirely.

Scheduling: TRN2's PE p-state throttle only reaches 2.4 GHz after 3us of
uninterrupted work, so the kernel keeps the PE queue dense. All deferrable
matmul work — Q-projection for heads 1-5 and the first two O-projection
passes (h0-2, h3-4, merged through a bf16 SBUF partial) — is enqueued as
closures and pumped one unit per chunk-iteration inside the attention
loops, filling what would otherwise be exp-wait bubbles. After head 5 only
a single-matmul O-pass remains: partial + h5 contribution accumulate in
PSUM via an identity matmul, with writebacks alternating ACT/DVE.
"""

import numpy as np
import ml_dtypes

HIDDEN = 1536
D = 128          # head dim
NH = 6           # q-heads per core
B, S = 2, 2048
SQ = 1024        # query tokens per core
HC = HIDDEN // 128   # 12 hidden chunks
SKC = S // 128       # 16 key chunks
SCALE = float(1.0 / np.sqrt(np.float32(D)))

_NC_CACHE = {}
last_results = None  # BassKernelResults of the most recent run (for test.py)


def _build_nc():
    import concourse.bacc as bacc
    import concourse.mybir as mybir
    import concourse.tile as tile
    from concourse.masks import make_identity

    bf16 = mybir.dt.bfloat16
    f32 = mybir.dt.float32
    Exp = mybir.ActivationFunctionType.Exp
    Copy = mybir.ActivationFunctionType.Copy

    nc = bacc.Bacc("TRN2", target_bir_lowering=False, debug=False, num_devices=8)

    xt = nc.dram_tensor("xt", [HIDDEN, S], bf16, kind="ExternalInput")
    wqt = nc.dram_tensor("wqt", [HIDDEN, NH * D], bf16, kind="ExternalInput")
    wkt = nc.dram_tensor("wkt", [HIDDEN, D], bf16, kind="ExternalInput")
    wvt = nc.dram_tensor("wvt", [HIDDEN, D], bf16, kind="ExternalInput")
    wot = nc.dram_tensor("wot", [NH * D, HIDDEN], bf16, kind="ExternalInput")
    biasd = nc.dram_tensor("biasd", [128, SKC], f32, kind="ExternalInput")
    y = nc.dram_tensor("y", [SQ, HIDDEN], bf16, kind="ExternalOutput")

    NB = HIDDEN // 512   # 3 o-proj column blocks
    NT = SQ // 128       # 8 o-proj token blocks
    NU = NT * NB         # 24 o-proj tiles

    with tile.TileContext(nc) as tc:
        with (
            tc.tile_pool(name="const", bufs=1) as constp,
            tc.tile_pool(name="weights", bufs=1) as wp,
            tc.tile_pool(name="persist", bufs=1) as pers,
        ):
            ident = constp.tile([128, 128], bf16)
            make_identity(nc, ident[:])
            ones_col = constp.tile([128, 1], bf16)
            nc.vector.memset(ones_col[:], 1.0)
            ones_row = constp.tile([1, 128], bf16)
            nc.vector.memset(ones_row[:], 1.0)
            bias_sb = constp.tile([128, SKC], f32)

            wk_sb = wp.tile([128, HC, D], bf16)
            wv_sb = wp.tile([128, HC, D], bf16)
            wo_sb = wp.tile([128, NH, HIDDEN], bf16)

            kT_sb = pers.tile([128, S], bf16)         # K^T [d, Sk]
            v_sb = pers.tile([128, SKC, D], bf16)     # V [Sk, d], chunked
            qT_sb = pers.tile([128, NH, SQ], bf16)    # Q^T [d, Sq] per head
            outT_sb = pers.tile([128, NH, SQ], bf16)  # normalized AV out^T
            part_sb = pers.tile([128, NU, 512], bf16) # o-proj partials
            xt_sb = pers.tile([128, HC, S], bf16)
            wq_sb = pers.tile([128, HC, NH * D], bf16)
            vT_sb = pers.tile([128, S], bf16)

            # DMA order is the early-pipeline schedule. xt streams in half-
            # chunks: the first K/V pass and ALL of Q-projection only touch
            # token columns 0-1023, so those halves come first; wq rides the
            # tail of the half-0 stream, then the half-1 columns, wo last.
            wqt_r = wqt.ap().rearrange("(c p) m -> p c m", p=128)
            nc.sync.dma_start(xt_sb[:, 0, 0:1024], xt[0:128, 0:1024])
            nc.sync.dma_start(wk_sb[:], wkt.ap().rearrange("(c p) m -> p c m", p=128))
            nc.sync.dma_start(wv_sb[:], wvt.ap().rearrange("(c p) m -> p c m", p=128))
            nc.sync.dma_start(bias_sb[:], biasd.ap())
            wq_queue = list(range(HC))
            for c in range(1, HC):
                nc.sync.dma_start(
                    xt_sb[:, c, 0:1024], xt[128 * c : 128 * (c + 1), 0:1024]
                )
                if c >= 4:
                    cq = wq_queue.pop(0)
                    nc.sync.dma_start(wq_sb[:, cq, :], wqt_r[:, cq, :])
            for cq in wq_queue[:2]:
                nc.sync.dma_start(wq_sb[:, cq, :], wqt_r[:, cq, :])
            wq_queue = wq_queue[2:]
            for c in range(HC):
                nc.sync.dma_start(
                    xt_sb[:, c, 1024:2048], xt[128 * c : 128 * (c + 1), 1024:2048]
                )
                if wq_queue:
                    cq = wq_queue.pop(0)
                    nc.sync.dma_start(wq_sb[:, cq, :], wqt_r[:, cq, :])
            nc.sync.dma_start(
                wo_sb[:], wot.ap().rearrange("(h p) n -> p h n", p=128)
            )

            # ---------- K/V + Q0 projection, V transpose ----------
            # K/V runs as two ping-ponged half-token passes (4 PSUM banks)
            # so this pool coexists with the Q0/transpose pool: the PE never
            # waits on a pool-boundary PSUM->SBUF cast drain.
            with (
                tc.tile_pool(name="kv_ps", bufs=1, space="PSUM") as kvps,
                tc.tile_pool(name="proj_ps", bufs=1, space="PSUM") as pps,
                tc.tile_pool(name="vtr_ps", bufs=2, space="PSUM") as vtrp,
            ):
                qps = pps.tile([128, SQ], f32, tag="projq")

                def kv_pass(p):
                    kv = kvps.tile([128, 2, 2, 512], f32, tag="kv")
                    for c in range(HC):
                        for ti, w_sb in ((0, wk_sb), (1, wv_sb)):
                            for sb in range(2):
                                nc.tensor.matmul(
                                    kv[:, ti, sb, :],
                                    w_sb[:, c, :],
                                    xt_sb[:, c, 512 * (2 * p + sb) : 512 * (2 * p + sb + 1)],
                                    start=(c == 0),
                                    stop=(c == HC - 1),
                                )
                    for ti, dst in ((1, vT_sb), (0, kT_sb)):
                        for sb in range(2):
                            blk = 2 * p + sb
                            nc.vector.tensor_copy(
                                dst[:, 512 * blk : 512 * (blk + 1)], kv[:, ti, sb, :]
                            )

                def q0_half(sqh):
                    for c in range(HC):
                        nc.tensor.matmul(
                            qps[:, 512 * sqh : 512 * (sqh + 1)],
                            wq_sb[:, c, 0:D],
                            xt_sb[:, c, 512 * sqh : 512 * (sqh + 1)],
                            start=(c == 0),
                            stop=(c == HC - 1),
                        )

                kv_pass(0)
                q0_half(0)
                kv_pass(1)
                q0_half(1)
                nc.vector.tensor_copy(qT_sb[:, 0, :], qps[:])
                for c in range(SKC):
                    pt = vtrp.tile([128, 128], bf16, tag="vtr")
                    nc.tensor.transpose(
                        pt[:], vT_sb[:, 128 * c : 128 * (c + 1)], ident[:]
                    )
                    nc.vector.tensor_copy(v_sb[:, c, :], pt[:])

            # ---------- attention + pipelined projections ----------
            with (
                tc.tile_pool(name="sc_ps", bufs=2, space="PSUM") as scp,
                tc.tile_pool(name="av_ps", bufs=1, space="PSUM") as avp,
                tc.tile_pool(name="y_ps", bufs=2, space="PSUM") as yp,
                tc.tile_pool(name="esb", bufs=8) as ep,
                tc.tile_pool(name="eaccp", bufs=2) as eaccp,
                tc.tile_pool(name="rowp", bufs=4) as rowp,
                tc.tile_pool(name="brecp", bufs=2) as brecp,
                tc.tile_pool(name="avsb", bufs=2) as avsbp,
                tc.tile_pool(name="y_sb", bufs=6) as ysb,
            ):
                # deferred PE work: ('q', head, fn) or ('o', fn)
                unit_q = []

                def pump(n):
                    for _ in range(min(n, len(unit_q))):
                        unit_q.pop(0)[-1]()

                def drain_q(h):
                    while unit_q and unit_q[0][0] == "q" and unit_q[0][1] <= h:
                        unit_q.pop(0)[-1]()

                def q_stream(qh, sqh):
                    def run():
                        ps = yp.tile([128, 512], f32, tag="y")
                        for c in range(HC):
                            nc.tensor.matmul(
                                ps[:],
                                wq_sb[:, c, D * qh : D * (qh + 1)],
                                xt_sb[:, c, 512 * sqh : 512 * (sqh + 1)],
                                start=(c == 0),
                                stop=(c == HC - 1),
                            )
                        nc.vector.tensor_copy(
                            qT_sb[:, qh, 512 * sqh : 512 * (sqh + 1)], ps[:]
                        )

                    return run

                def opass_unit(u, heads, first):
                    t, nb = divmod(u, NB)

                    def run():
                        ps = yp.tile([128, 512], f32, tag="y")
                        for i, h in enumerate(heads):
                            nc.tensor.matmul(
                                ps[:],
                                outT_sb[:, h, 128 * t : 128 * (t + 1)],
                                wo_sb[:, h, 512 * nb : 512 * (nb + 1)],
                                start=(i == 0),
                                stop=(i == len(heads) - 1),
                            )
                        if first:
                            nc.vector.tensor_copy(part_sb[:, u, :], ps[:])
                        else:
                            nc.vector.tensor_add(
                                part_sb[:, u, :], part_sb[:, u, :], ps[:]
                            )

                    return run

                for qh in range(1, NH):
                    for sqh in range(2):
                        unit_q.append(("q", qh, q_stream(qh, sqh)))

                for h in range(NH):
                    drain_q(h)
                    av = avp.tile([128, SQ], f32, tag="av")
                    eacc = eaccp.tile([128, SQ], bf16, tag="eacc")
                    e_tiles = {}

                    def emit_scores(c):
                        # scores^T chunk [Sk 128, Sq 1024] = (K^T slice)^T Q^T
                        sc = scp.tile([128, SQ], f32, tag="sc")
                        for sqh in range(2):
                            nc.tensor.matmul(
                                sc[:, 512 * sqh : 512 * (sqh + 1)],
                                kT_sb[:, 128 * c : 128 * (c + 1)],
                                qT_sb[:, h, 512 * sqh : 512 * (sqh + 1)],
                                start=True,
                                stop=True,
                            )
                        et = ep.tile([128, SQ], bf16, tag="e")
                        nc.scalar.activation(
                            et[:], sc[:], Exp,
                            bias=bias_sb[:, c : c + 1], scale=SCALE,
                        )
                        e_tiles[c] = et

                    def emit_av_acc(c):
                        et = e_tiles[c]
                        for sqh in range(2):
                            nc.tensor.matmul(
                                av[:, 512 * sqh : 512 * (sqh + 1)],
                                v_sb[:, c, :],
                                et[:, 512 * sqh : 512 * (sqh + 1)],
                                start=(c == 0),
                                stop=(c == SKC - 1),
                            )
                        # denominator accumulation stays off the PE; the last
                        # chunk is folded into the reduction matmul instead
                        if c == 0:
                            nc.vector.tensor_copy(eacc[:], et[:])
                        elif c < SKC - 1:
                            nc.vector.tensor_add(eacc[:], eacc[:], et[:])

                    emit_scores(0)
                    for c in range(SKC):
                        if c + 1 < SKC:
                            emit_scores(c + 1)
                        emit_av_acc(c)
                        pump(2 if h >= 4 else 1)

                    # head tail: ones-matmul partition reduction of the
                    # denominator (+ last exp chunk), rank-1 broadcast,
                    # reciprocal + normalize. ACT copies av out of PSUM so
                    # the next head's AV accumulation isn't gated on this.
                    e15 = e_tiles[SKC - 1]
                    av_f = avsbp.tile([128, SQ], f32, tag="avf")
                    nc.scalar.activation(av_f[:], av[:], Copy)
                    for sqh in range(2):
                        sl = slice(512 * sqh, 512 * (sqh + 1))
                        rs = yp.tile([128, 512], f32, tag="y")
                        nc.tensor.matmul(
                            rs[0:1, :], ones_col[:], eacc[:, sl],
                            start=True, stop=False,
                        )
                        nc.tensor.matmul(
                            rs[0:1, :], ones_col[:], e15[:, sl],
                            start=False, stop=True,
                        )
                        row = rowp.tile([1, 512], bf16, tag="row")
                        nc.vector.tensor_copy(row[:], rs[0:1, :])
                        bc = yp.tile([128, 512], f32, tag="y")
                        nc.tensor.matmul(
                            bc[:], ones_row[:], row[:], start=True, stop=True
                        )
                        brec = brecp.tile([128, 512], f32, tag="brec")
                        nc.vector.reciprocal_approx_fast(brec[:], bc[:])
                        nc.vector.tensor_mul(
                            outT_sb[:, h, sl], av_f[:, sl], brec[:]
                        )

                    if h == 2:
                        for u in range(NU):
                            unit_q.append(("o", opass_unit(u, (0, 1, 2), True)))
                    elif h == 4:
                        for u in range(NU):
                            unit_q.append(("o", opass_unit(u, (3, 4), False)))
                    # keep the PE fed while the normalization chain resolves
                    pump(2)

                # drain leftovers, then the final o-pass: bf16 partial folded
                # back into PSUM via an identity matmul + h5's contribution.
                # The sc/av pools are dead now, so pass-3 tiles spread over
                # all 8 PSUM banks (8 units in flight) and writebacks
                # alternate ACT/DVE so neither engine serializes the drain.
                pump(len(unit_q))

                p3_cache = {}

                def p3_tile(u):
                    k = u % 4
                    if k == 0:
                        t = scp.tile([128, SQ], f32, tag="sc")
                        p3_cache["t"] = t
                        return t[:, 0:512]
                    if k == 1:
                        return p3_cache["t"][:, 512:]
                    if k == 2:
                        t = avp.tile([128, SQ], f32, tag="av")
                        p3_cache["t"] = t
                        return t[:, 0:512]
                    return p3_cache["t"][:, 512:]

                for u in range(NU):
                    t, nb = divmod(u, NB)
                    ps = p3_tile(u)
                    nc.tensor.matmul(
                        ps, ident[:], part_sb[:, u, :], start=True, stop=False
                    )
                    nc.tensor.matmul(
                        ps,
                        outT_sb[:, 5, 128 * t : 128 * (t + 1)],
                        wo_sb[:, 5, 512 * nb : 512 * (nb + 1)],
                        start=False,
                        stop=True,
                    )
                    ysb_t = ysb.tile([128, 512], bf16, tag="ysb")
                    if u % 2 == 0:
                        nc.scalar.activation(ysb_t[:], ps, Copy)
                    else:
                        nc.vector.tensor_copy(ysb_t[:], ps)
                    nc.sync.dma_start(
                        y[128 * t : 128 * (t + 1), 512 * nb : 512 * (nb + 1)],
                        ysb_t[:],
                    )

    nc.compile()
    return nc


def _get_nc():
    if "nc" not in _NC_CACHE:
        _NC_CACHE["nc"] = _build_nc()
    return _NC_CACHE["nc"]


def kernel(hidden_states, attention_mask, Wq, Wk, Wv, Wo):
    global last_results
    from concourse.bass_utils import run_bass_kernel_spmd

    bf = ml_dtypes.bfloat16
    hidden_states = np.asarray(hidden_states, dtype=np.float32)
    attention_mask = np.asarray(attention_mask, dtype=np.float32)
    Wq = np.asarray(Wq, dtype=np.float32)
    Wk = np.asarray(Wk, dtype=np.float32)
    Wv = np.asarray(Wv, dtype=np.float32)
    Wo = np.asarray(Wo, dtype=np.float32)

    nc = _get_nc()

    in_maps = []
    cores = []
    for b in range(2):
        xt_full = np.ascontiguousarray(hidden_states[b].T).astype(bf)  # [H, S]
        bias_full = ((1.0 - attention_mask[b]) * -10000.0).astype(np.float32)
        for hh in range(2):
            wqt = np.ascontiguousarray(
                Wq[NH * D * hh : NH * D * (hh + 1), :].T
            ).astype(bf)
            wkt = np.ascontiguousarray(Wk[D * hh : D * (hh + 1), :].T).astype(bf)
            wvt = np.ascontiguousarray(Wv[D * hh : D * (hh + 1), :].T).astype(bf)
            wot = np.ascontiguousarray(
                Wo[:, NH * D * hh : NH * D * (hh + 1)].T
            ).astype(bf)
            for th in range(2):
                # roll tokens so this core's queries are columns 0..SQ-1
                r = th * SQ
                xt_r = np.ascontiguousarray(
                    np.concatenate([xt_full[:, r:], xt_full[:, :r]], axis=1)
                )
                bias_r = np.concatenate([bias_full[r:], bias_full[:r]])
                biasd = np.ascontiguousarray(
                    bias_r.reshape(SKC, 128).T
                ).astype(np.float32)
                in_maps.append(
                    {
                        "xt": xt_r,
                        "wqt": wqt,
                        "wkt": wkt,
                        "wvt": wvt,
                        "wot": wot,
                        "biasd": biasd,
                    }
                )
                cores.append((b, hh, th))

    res = run_bass_kernel_spmd(nc, in_maps, core_ids=list(range(8)))
    last_results = res

    out = np.zeros((B, S, HIDDEN), dtype=np.float32)
    for (b, hh, th), r in zip(cores, res.results):
        out[b, th * SQ : (th + 1) * SQ, :] += np.asarray(r["y"], dtype=np.float32)
    return out
